# revision 1
# baseline (speedup 1.0000x reference)
"""BertAlibiLayer on 8 TRN2 NeuronCores — data-parallel over batch.

Layout strategy: all activations on-chip are FEATURE-major ([feature, token]),
which makes every matmul transpose-free (weights are pre-transposed on host).
Attention computes scoresT = [key, query]; softmax normalization comes from a
ones-column folded into V (denominator lands as a psum row) and is applied as
exp(-ln(denom)) broadcast across partitions by GPSIMD. LayerNorm reductions
(over features = partitions) use ones-vector matmuls on the PE; mean/rstd are
broadcast back across partitions by GPSIMD into SBUF.

Per core: 2 sequences x 512 tokens (N=1024 token-columns), full weights.
Projection/MLP matmuls run as float32r (full-rate fp32 streaming); the
attention probs path and the Wdown contraction use bf16.

PSUM pools (4+2+2 banks) stay open for the whole kernel so no phase ever
serializes on bank reuse.
"""

from contextlib import ExitStack

import numpy as np
import ml_dtypes

import concourse.bass as bass
import concourse.mybir as mybir
import concourse.tile as tile
from concourse import bacc
from concourse.bass_utils import run_bass_kernel_spmd

F32 = mybir.dt.float32
F32R = mybir.dt.float32r
BF16 = mybir.dt.bfloat16
AF = mybir.ActivationFunctionType
OP = mybir.AluOpType

DIM = 768
H = 12
HD = 64
S = 512
NSEQ = 2          # sequences per core
N = NSEQ * S      # tokens per core
I = 3072
KT = DIM // 128   # 6 k-tiles over DIM
EPS = 1e-12
N_CORES = 8

OC_ORDER = [0, 6, 1, 7, 2, 8, 3, 9, 4, 10, 5, 11]  # q/k chunk emission order


def r(ap):
    """View an fp32 AP as float32r for full-rate PE streaming."""
    return ap.bitcast(F32R)


def build_program(gelu_func=AF.Gelu):
    nc = bacc.Bacc("TRN2", target_bir_lowering=False, debug=False,
                   enable_asserts=False)
    # Steer the act-table chooser: the plain natural_log set lacks exp, so
    # Ln<->Exp sequences would reload tables every op. Emptying it (in place,
    # preserving set ids) makes the chooser use natural_log_exp_and_others.
    import concourse.hw_specs as hw_specs
    tabs = hw_specs.get_activation_tables(nc.m.arch)
    tabs["natural_log"] = set()

    # ---- DRAM parameters (per-core shards / replicated weights) ----
    xT = nc.dram_tensor("xT", [DIM, N], F32R, kind="ExternalInput").ap()
    # exp(bias)^T: softmax uses exp(s+b) = exp(s)*exp(b), exp(b) from host
    expbT = nc.dram_tensor("expbT", [NSEQ, H, S, S], BF16, kind="ExternalInput").ap()
    wqkvT = nc.dram_tensor("wqkvT", [DIM, 3 * DIM], F32R, kind="ExternalInput").ap()
    bqk = nc.dram_tensor("bqk", [128, 12], F32, kind="ExternalInput").ap()
    bv_b = nc.dram_tensor("bv_b", [128, DIM], BF16, kind="ExternalInput").ap()
    woT = nc.dram_tensor("woT", [DIM, DIM], F32R, kind="ExternalInput").ap()
    bo = nc.dram_tensor("bo", [128, 6], F32, kind="ExternalInput").ap()
    wgluT = nc.dram_tensor("wgluT", [DIM, 2 * I], F32R, kind="ExternalInput").ap()
    cb1 = nc.dram_tensor("cb1", [128, 48], F32, kind="ExternalInput").ap()
    g1 = nc.dram_tensor("g1", [128, 6], F32, kind="ExternalInput").ap()
    c1 = nc.dram_tensor("c1", [128, 6], F32, kind="ExternalInput").ap()
    wdownT = nc.dram_tensor("wdownT", [I, DIM], BF16, kind="ExternalInput").ap()
    g2 = nc.dram_tensor("g2", [128, 6], F32, kind="ExternalInput").ap()
    b2 = nc.dram_tensor("b2", [128, 6], F32, kind="ExternalInput").ap()
    outT = nc.dram_tensor("outT", [DIM, N], F32, kind="ExternalOutput").ap()

    with tile.TileContext(nc) as tc:
        emit(nc, tc, xT, expbT, wqkvT, bqk, bv_b, woT, bo, wgluT, cb1, g1, c1,
             wdownT, g2, b2, outT, gelu_func)

    nc.compile()
    return nc


def emit(nc, tc, xT, expbT, wqkvT, bqk, bv_b, woT, bo, wgluT, cb1, g1, c1,
         wdownT, g2, b2, outT, gelu_func=AF.Gelu):
    root = ExitStack()
    consts = root.enter_context(tc.tile_pool(name="consts", bufs=1, side="left"))
    # PSUM pools — opened once for the whole kernel (8 banks total) so no
    # phase transition ever waits on bank reuse.
    pmm = root.enter_context(tc.tile_pool(name="pmm", bufs=4, space="PSUM"))
    pstat = root.enter_context(tc.tile_pool(name="pstat", bufs=2, space="PSUM"))
    pbc = root.enter_context(tc.tile_pool(name="pbc", bufs=2, space="PSUM"))

    # ---------------- Phase 1: QKV projection ----------------
    xt_ctx = ExitStack()
    xt_pool = xt_ctx.enter_context(tc.tile_pool(name="xt", bufs=KT, side="right"))
    qkva_ctx = ExitStack()
    qk_pool = qkva_ctx.enter_context(tc.tile_pool(name="qk", bufs=24, side="left"))
    va_pool = qkva_ctx.enter_context(tc.tile_pool(name="vaug", bufs=8, side="left"))
    p1_ctx = ExitStack()
    wq_pool = p1_ctx.enter_context(tc.tile_pool(name="wqkv", bufs=6, side="left"))

    # Critical-path DMAs first: the very first matmul chain (oc=0, half 0)
    # needs chunks (kt, 0) and xt half-0 only (~2MB), so those dispatch first.
    wqk_sb = [[None] * 12 for _ in range(KT)]
    xt_sb = []
    for kt in range(KT):
        c = wq_pool.tile([128, 128], F32R, name=f"wqk{kt}_0", tag="wqk",
                         bufs=72)
        nc.sync.dma_start(c[:], wqkvT[kt * 128:(kt + 1) * 128, 0:128])
        wqk_sb[kt][0] = c
    for kt in range(KT):
        t = xt_pool.tile([128, N], F32R, name=f"xt{kt}", tag="xt")
        nc.sync.dma_start(t[:, 0:512], xT[kt * 128:(kt + 1) * 128, 0:512])
        xt_sb.append(t)
    for kt in range(KT):
        oc = 6
        c = wq_pool.tile([128, 128], F32R, name=f"wqk{kt}_6", tag="wqk",
                         bufs=72)
        nc.sync.dma_start(c[:], wqkvT[kt * 128:(kt + 1) * 128,
                                      oc * 128:(oc + 1) * 128])
        wqk_sb[kt][6] = c

    # small constant tensors
    bqk_sb = consts.tile([128, 12], F32)
    nc.sync.dma_start(bqk_sb[:], bqk[:, :])
    bvb_sb = consts.tile([128, DIM], BF16)
    nc.sync.dma_start(bvb_sb[:], bv_b[:, :])
    bo_sb = consts.tile([128, 6], F32)
    nc.sync.dma_start(bo_sb[:], bo[:, :])
    cb1_sb = consts.tile([128, 48], F32)
    nc.sync.dma_start(cb1_sb[:], cb1[:, :])
    g1_sb = consts.tile([128, 6], F32)
    nc.sync.dma_start(g1_sb[:], g1[:, :])
    c1_sb = consts.tile([128, 6], F32)
    nc.sync.dma_start(c1_sb[:], c1[:, :])
    g2_sb = consts.tile([128, 6], F32)
    nc.sync.dma_start(g2_sb[:], g2[:, :])
    b2_sb = consts.tile([128, 6], F32)
    nc.sync.dma_start(b2_sb[:], b2[:, :])
    # f32 ones staging (memset cannot write f32r; DVE copies round instead)
    ones_f32c = consts.tile([128, 12], F32)
    nc.vector.memset(ones_f32c[:], 1.0)
    ones_col = consts.tile([128, 1], F32)   # stats lhsT: column of ones
    nc.vector.tensor_copy(ones_col[:].bitcast(F32R), ones_f32c[:, 0:1])
    ones_row = consts.tile([1, 128], F32)   # LN broadcast lhsT: row of ones
    nc.vector.memset(ones_row[:], 1.0)
    nc.vector.tensor_copy(ones_row[:].bitcast(F32R), ones_row[:])
    eps_sb = consts.tile([1, 1], F32)
    nc.vector.memset(eps_sb[:], EPS)

    # remaining x columns + weight chunks in chain-consumption order
    for kt in range(KT):
        nc.sync.dma_start(xt_sb[kt][:, 512:1024],
                          xT[kt * 128:(kt + 1) * 128, 512:1024])
    for oc in OC_ORDER[2:]:
        for kt in range(KT):
            c = wq_pool.tile([128, 128], F32R, name=f"wqk{kt}_{oc}", tag="wqk",
                             bufs=72)
            nc.sync.dma_start(c[:], wqkvT[kt * 128:(kt + 1) * 128,
                                          oc * 128:(oc + 1) * 128])
            wqk_sb[kt][oc] = c
    wv_sb = []
    for kt in range(KT):
        v = wq_pool.tile([128, DIM], F32R, name=f"wv{kt}", tag="wv", bufs=6)
        nc.sync.dma_start(v[:], wqkvT[kt * 128:(kt + 1) * 128, 2 * DIM:])
        wv_sb.append(v)

    # q,k feature-major, one PADDED tile per head: rows 0..63 = head features,
    # rows 64..127 zeroed so the scores matmul runs at K=128 with fast weight
    # load (an isolated K=64 matmul costs ~2x). qk_sb[0:12]=q, [12:24]=k.
    qk_sb = [None] * 24
    for oc in OC_ORDER:
        base = (oc - 6 + 12) if oc >= 6 else oc   # tile index of first head
        ha = 2 * (oc % 6) + (12 if oc >= 6 else 0)
        t0 = qk_pool.tile([128, N], BF16, name=f"qkh{ha}", tag="qk")
        t1 = qk_pool.tile([128, N], BF16, name=f"qkh{ha + 1}", tag="qk")
        qk_sb[ha] = t0
        qk_sb[ha + 1] = t1
        nc.vector.memset(t0[64:128, :], 0.0)
        nc.vector.memset(t1[64:128, :], 0.0)
        for h2 in range(2):
            ps = pmm.tile([128, 512], F32, tag="ps")
            for kt in range(KT):
                nc.tensor.matmul(
                    ps[:], wqk_sb[kt][oc][:],
                    r(xt_sb[kt][:, h2 * 512:(h2 + 1) * 512]),
                    start=(kt == 0), stop=(kt == KT - 1),
                )
            nc.scalar.activation(t0[0:64, h2 * 512:(h2 + 1) * 512],
                                 ps[0:64, :], AF.Identity,
                                 bias=bqk_sb[0:64, oc:oc + 1])
            nc.scalar.activation(t1[0:64, h2 * 512:(h2 + 1) * 512],
                                 ps[64:128, :], AF.Identity,
                                 bias=bqk_sb[64:128, oc:oc + 1])

    # v in natural token-major layout; each head padded to a 128-wide block
    # (col 64 = ones -> softmax denominator; cols 65..127 zero) so the ctx
    # matmul's stationary operand is a full 128-column bf16 tile (fast FWL).
    va_sb = []
    for sc in range(8):
        vt = va_pool.tile([128, H * 128], BF16, name=f"vaug{sc}", tag="vaug")
        va_sb.append(vt)
        vt_h = vt[:].rearrange("p (h c) -> p h c", c=128)
        nc.vector.memset(vt_h[:, :, HD + 1:], 0.0)
        nc.vector.tensor_copy(vt_h[:, :, HD:HD + 1],
                              ones_f32c[:].rearrange("p (h c) -> p h c", c=1))
        for off, width, h0 in ((0, 512, 0), (512, 256, 8)):
            nh = width // HD
            ps = pmm.tile([128, 512], F32, tag="ps")
            for kt in range(KT):
                nc.tensor.matmul(
                    ps[:, :width],
                    r(xt_sb[kt][:, sc * 128:(sc + 1) * 128]),
                    r(wv_sb[kt][:, off:off + width]),
                    start=(kt == 0), stop=(kt == KT - 1),
                )
            nc.vector.tensor_add(
                vt_h[:, h0:h0 + nh, 0:HD],
                ps[:, :width].rearrange("p (h c) -> p h c", c=HD),
                bvb_sb[:, off:off + width].rearrange("p (h c) -> p h c", c=HD),
            )
    p1_ctx.close()

    # ---------------- Phase 2: attention (per sequence, per head) ----------
    ctx_ctx = ExitStack()
    ctx_pool = ctx_ctx.enter_context(tc.tile_pool(name="ctxT", bufs=12, side="right"))
    ctx_sb = [ctx_pool.tile([128, 512], F32, name=f"ctx{i}", tag="ctx")
              for i in range(NSEQ * KT)]

    p2_ctx = ExitStack()
    pb_pool = p2_ctx.enter_context(tc.tile_pool(name="pbias", bufs=6, side="left"))
    sin_pool = p2_ctx.enter_context(tc.tile_pool(name="sin", bufs=6, side="left"))
    exp_pool = p2_ctx.enter_context(tc.tile_pool(name="exp", bufs=10, side="left"))
    rec_pool = p2_ctx.enter_context(tc.tile_pool(name="recip", bufs=2, side="left"))
    bcs_pool = p2_ctx.enter_context(tc.tile_pool(name="bcs", bufs=4, side="left"))

    for seq in range(NSEQ):
        for h in range(H):
            q_tile = qk_sb[h]
            k_tile = qk_sb[12 + h]
            # one batched DMA for all 4 key-chunks of this (seq, head)
            bt = pb_pool.tile([128, 4, 512], BF16, name=f"bt{seq}_{h}",
                              tag="bias")
            nc.gpsimd.dma_start(
                bt[:], expbT[seq, h].rearrange("(c p) i -> p c i", p=128))
            e_tiles = []
            for jt in range(4):
                ps = pmm.tile([128, 512], F32, tag="ps")
                nc.tensor.matmul(
                    ps[:],
                    k_tile[:, seq * 512 + jt * 128:seq * 512 + (jt + 1) * 128],
                    q_tile[:, seq * 512:(seq + 1) * 512],
                    start=True, stop=True,
                )
                st = sin_pool.tile([128, 512], BF16, tag="sin")
                nc.scalar.activation(st[:], ps[:], AF.Exp)
                et = exp_pool.tile([128, 512], BF16, tag="exp")
                nc.vector.tensor_mul(et[:], st[:], bt[:, jt, :])
                e_tiles.append(et)

            pc = pmm.tile([128, 512], F32, tag="ps")
            for jt in range(4):
                nc.tensor.matmul(
                    pc[:],
                    va_sb[seq * 4 + jt][:, h * 128:h * 128 + 128],
                    e_tiles[jt][:],
                    start=(jt == 0), stop=(jt == 3),
                )
            ld = rec_pool.tile([1, 512], F32, tag="ln")
            nc.scalar.activation(ld[:], pc[HD:HD + 1, :], AF.Ln)
            rc = rec_pool.tile([1, 512], F32, tag="recip")
            nc.scalar.activation(rc[:], ld[:], AF.Exp, scale=-1.0)
            bc = bcs_pool.tile([64, 512], F32, tag="bc")
            nc.gpsimd.partition_broadcast(bc[:], rc[:], channels=64)
            nc.vector.tensor_mul(
                ctx_sb[seq * KT + h // 2][(h % 2) * 64:(h % 2) * 64 + 64, :]
                .bitcast(F32R),
                pc[0:HD, :], bc[:])
    p2_ctx.close()
    qkva_ctx.close()

    # ---------------- Phase 3: Wo projection + residual -------------------
    s1_ctx = ExitStack()
    s1_pool = s1_ctx.enter_context(tc.tile_pool(name="s1", bufs=KT, side="left"))
    s1_sb = [s1_pool.tile([128, N], F32, name=f"s1_{oc}", tag="s1")
             for oc in range(KT)]

    p3_ctx = ExitStack()
    wo_pool = p3_ctx.enter_context(tc.tile_pool(name="wo", bufs=KT, side="left"))
    wo_sb = []
    for kt in range(KT):
        t = wo_pool.tile([128, DIM], F32R, name=f"wo{kt}", tag="wo")
        nc.sync.dma_start(t[:], woT[kt * 128:(kt + 1) * 128, :])
        wo_sb.append(t)

    for seq in range(NSEQ):
        for oc in range(KT):
            ps = pmm.tile([128, 512], F32, tag="ps")
            for kt in range(KT):
                nc.tensor.matmul(
                    ps[:],
                    r(wo_sb[kt][:, oc * 128:(oc + 1) * 128]),
                    r(ctx_sb[seq * KT + kt][:]),
                    start=(kt == 0), stop=(kt == KT - 1),
                )
            # s1 = wo_out + bo + x   (attention residual)
            nc.vector.scalar_tensor_tensor(
                s1_sb[oc][:, seq * 512:(seq + 1) * 512].bitcast(F32R),
                ps[:], bo_sb[:, oc:oc + 1],
                xt_sb[oc][:, seq * 512:(seq + 1) * 512].bitcast(F32),
                op0=OP.add, op1=OP.add,
            )
    p3_ctx.close()
    ctx_ctx.close()
    xt_ctx.close()

    # GLU weight stream: pool opened and DMAs emitted BEFORE LayerNorm 1 so
    # the SP queue dispatches them at attention end (they reuse the dead
    # wo/attention-transient zones, not s1's) and the og0 weights are resident
    # when the first glu matmul unblocks.
    wg_ctx = ExitStack()
    wg_pool = wg_ctx.enter_context(tc.tile_pool(name="wglu", bufs=12, side="left"))
    wg1s = []
    wg2s = []
    for g in range(3):
        wg1s.append([])
        wg2s.append([])
        for kt in range(KT):
            t = wg_pool.tile([128, 1024], F32R, name=f"wg1_{g}_{kt}", tag="wg")
            nc.sync.dma_start(
                t[:], wgluT[kt * 128:(kt + 1) * 128, g * 1024:(g + 1) * 1024])
            wg1s[g].append(t)
            t2 = wg_pool.tile([128, 1024], F32R, name=f"wg2_{g}_{kt}", tag="wg")
            nc.sync.dma_start(
                t2[:], wgluT[kt * 128:(kt + 1) * 128,
                             I + g * 1024:I + (g + 1) * 1024])
            wg2s[g].append(t2)

    # ---------------- shared LayerNorm helper ------------------------------
    def layernorm(src_sb, dst_cb, sq_pool, stat_pool):
        """Feature-axis layernorm over KT source tiles [128, N]. Stats via
        ones-matmuls; mean/rstd broadcast across partitions by GPSIMD into
        SBUF; dst_cb(oc, half, mbc, rbc) applies."""
        for half in range(2):
            hs = slice(half * 512, (half + 1) * 512)
            psx_t = pstat.tile([1, 512], F32, tag="st", name="psx")
            psxx_t = pstat.tile([1, 512], F32, tag="st", name="psxx")
            psx = psx_t[:]
            psxx = psxx_t[:]

            for oc in range(KT):
                sq = sq_pool.tile([128, 512], F32, tag="sq")
                nc.scalar.activation(sq[:].bitcast(F32R), src_sb[oc][:, hs],
                                     AF.Square)
                nc.tensor.matmul(psx, r(ones_col[:]), r(src_sb[oc][:, hs]),
                                 start=(oc == 0), stop=(oc == KT - 1))
                nc.tensor.matmul(psxx, r(ones_col[:]), r(sq[:]),
                                 start=(oc == 0), stop=(oc == KT - 1))
            m_sb = stat_pool.tile([1, 512], F32, tag="st")
            nc.scalar.activation(m_sb[:], psx, AF.Identity, scale=1.0 / DIM)
            msq = stat_pool.tile([1, 512], F32, tag="st")
            nc.scalar.activation(msq[:], psx, AF.Square, scale=1.0 / DIM)
            var = stat_pool.tile([1, 512], F32, tag="st")
            nc.vector.scalar_tensor_tensor(var[:], psxx, 1.0 / DIM, msq[:],
                                           op0=OP.mult, op1=OP.subtract)
            lv = stat_pool.tile([1, 512], F32, tag="st")
            nc.scalar.activation(lv[:], var[:], AF.Ln, bias=eps_sb[:1, :1])
            rs = stat_pool.tile([1, 512], F32, tag="st")
            nc.scalar.activation(rs[:], lv[:], AF.Exp, scale=-0.5)
            # rounding copies so the K=1 broadcast matmuls see f32r inputs
            m_r = stat_pool.tile([1, 512], F32, tag="st")
            nc.vector.tensor_copy(m_r[:].bitcast(F32R), m_sb[:])
            rs_r = stat_pool.tile([1, 512], F32, tag="st")
            nc.vector.tensor_copy(rs_r[:].bitcast(F32R), rs[:])
            mbc = pbc.tile([128, 512], F32, tag="bc")
            nc.tensor.matmul(mbc[:], r(ones_row[:]), r(m_r[:]),
                             start=True, stop=True)
            rbc = pbc.tile([128, 512], F32, tag="bc")
            nc.tensor.matmul(rbc[:], r(ones_row[:]), r(rs_r[:]),
                             start=True, stop=True)
            for oc in range(KT):
                dst_cb(oc, half, mbc, rbc)

    # ---------------- Phase 4: LayerNorm 1 --------------------------------
    rz_ctx = ExitStack()
    r_pool = rz_ctx.enter_context(tc.tile_pool(name="resid", bufs=KT, side="right"))
    r_sb = [r_pool.tile([128, N], F32, name=f"r{oc}", tag="resid")
            for oc in range(KT)]
    z1_ctx = ExitStack()
    z1_pool = z1_ctx.enter_context(tc.tile_pool(name="z1", bufs=KT, side="right"))
    z1_sb = [z1_pool.tile([128, N], F32, name=f"z1_{oc}", tag="z1")
             for oc in range(KT)]

    p4_ctx = ExitStack()
    sq1_pool = p4_ctx.enter_context(tc.tile_pool(name="sq1", bufs=4, side="left"))
    st1_pool = p4_ctx.enter_context(tc.tile_pool(name="st1", bufs=4, side="left"))
    t1_pool = p4_ctx.enter_context(tc.tile_pool(name="t1", bufs=3, side="left"))

    def ln1_apply(oc, half, mbc, rbc):
        hs = slice(half * 512, (half + 1) * 512)
        t = t1_pool.tile([128, 512], F32, tag="t")
        nc.vector.tensor_sub(t[:], s1_sb[oc][:, hs], mbc[:])
        nc.vector.tensor_mul(z1_sb[oc][:, hs].bitcast(F32R), t[:], rbc[:])

    layernorm(s1_sb, ln1_apply, sq1_pool, st1_pool)
    # residual carry: r = z1*g1 + (ln1_b + bdown)
    for oc in range(KT):
        nc.scalar.activation(r_sb[oc][:], z1_sb[oc][:], AF.Identity,
                             bias=c1_sb[:, oc:oc + 1], scale=g1_sb[:, oc:oc + 1])
    p4_ctx.close()

    # ---------------- Phase 5: GLU (gate = gelu(glu1+cb), mult branch) ----
    gated_ctx = ExitStack()
    gated_pool = gated_ctx.enter_context(
        tc.tile_pool(name="gated", bufs=24, side="left"))
    gated_sb = [gated_pool.tile([128, N], BF16, name=f"gated{j}", tag="gated")
                for j in range(24)]

    # first Wdown tiles stream during the GLU phase so the Wdown matmuls can
    # start the moment the last gated tile lands
    wde_ctx = ExitStack()
    wde_pool = wde_ctx.enter_context(tc.tile_pool(name="wde", bufs=6, side="left"))
    wd_sb = [None] * 24
    for kt in range(6):
        t = wde_pool.tile([128, DIM], BF16, name=f"wd{kt}", tag="wde")
        nc.sync.dma_start(t[:], wdownT[kt * 128:(kt + 1) * 128, :])
        wd_sb[kt] = t

    p5_ctx = ExitStack()
    gelu_pool = p5_ctx.enter_context(tc.tile_pool(name="gelu", bufs=4, side="left"))

    for g in range(3):
        wg1 = wg1s[g]
        wg2 = wg2s[g]
        for j in range(8):
            oc = g * 8 + j          # gate chunk index in [0, 24)
            for half in range(2):
                hs = slice(half * 512, (half + 1) * 512)
                ps = pmm.tile([128, 512], F32, tag="ps")
                for kt in range(KT):
                    nc.tensor.matmul(
                        ps[:], r(wg1[kt][:, j * 128:(j + 1) * 128]),
                        r(z1_sb[kt][:, hs]),
                        start=(kt == 0), stop=(kt == KT - 1))
                ge = gelu_pool.tile([128, 512], F32, tag="gelu")
                nc.scalar.activation(ge[:], ps[:], gelu_func,
                                     bias=cb1_sb[:, oc:oc + 1])
                ps2 = pmm.tile([128, 512], F32, tag="ps")
                for kt in range(KT):
                    nc.tensor.matmul(
                        ps2[:], r(wg2[kt][:, j * 128:(j + 1) * 128]),
                        r(z1_sb[kt][:, hs]),
                        start=(kt == 0), stop=(kt == KT - 1))
                # gated = (glu2 + cb1_2) * gelu(glu1 + cb1_1), stored bf16
                nc.vector.scalar_tensor_tensor(
                    gated_sb[oc][:, hs], ps2[:], cb1_sb[:, 24 + oc:25 + oc],
                    ge[:], op0=OP.add, op1=OP.mult)
    p5_ctx.close()
    z1_ctx.close()

    # ---------------- Phase 6: Wdown + residual ----------------------------
    s2_ctx = ExitStack()
    s2_pool = s2_ctx.enter_context(tc.tile_pool(name="s2", bufs=KT, side="left"))
    s2_sb = [s2_pool.tile([128, N], F32, name=f"s2_{oc}", tag="s2")
             for oc in range(KT)]
    p6_ctx = ExitStack()
    wd_pool = p6_ctx.enter_context(tc.tile_pool(name="wdown", bufs=18, side="left"))
    for kt in range(6, 24):
        t = wd_pool.tile([128, DIM], BF16, name=f"wd{kt}", tag="wd")
        nc.sync.dma_start(t[:], wdownT[kt * 128:(kt + 1) * 128, :])
        wd_sb[kt] = t

    for half in range(2):
        hs = slice(half * 512, (half + 1) * 512)
        for oc in range(KT):
            ps = pmm.tile([128, 512], F32, tag="ps")
            for kt in range(24):
                nc.tensor.matmul(
                    ps[:], wd_sb[kt][:, oc * 128:(oc + 1) * 128],
                    gated_sb[kt][:, hs],
                    start=(kt == 0), stop=(kt == 23))
            nc.vector.tensor_add(s2_sb[oc][:, hs].bitcast(F32R), ps[:],
                                 r_sb[oc][:, hs])
    p6_ctx.close()
    rz_ctx.close()

    # ---------------- Phase 7: LayerNorm 2 + output ------------------------
    p7_ctx = ExitStack()
    out_pool = p7_ctx.enter_context(tc.tile_pool(name="outp", bufs=8, side="right"))
    sq2_pool = p7_ctx.enter_context(tc.tile_pool(name="sq2", bufs=4, side="right"))
    st2_pool = p7_ctx.enter_context(tc.tile_pool(name="st2", bufs=4, side="right"))
    t2_pool = p7_ctx.enter_context(tc.tile_pool(name="t2", bufs=3, side="right"))

    def ln2_apply(oc, half, mbc, rbc):
        hs = slice(half * 512, (half + 1) * 512)
        t = t2_pool.tile([128, 512], F32, tag="t")
        nc.vector.tensor_sub(t[:], s2_sb[oc][:, hs], mbc[:])
        zz = t2_pool.tile([128, 512], F32, tag="zz")
        nc.vector.scalar_tensor_tensor(zz[:], t[:], g2_sb[:, oc:oc + 1],
                                       rbc[:], op0=OP.mult, op1=OP.mult)
        ot = out_pool.tile([128, 512], F32, tag="out")
        nc.scalar.activation(ot[:], zz[:], AF.Identity,
                             bias=b2_sb[:, oc:oc + 1])
        nc.sync.dma_start(outT[oc * 128:(oc + 1) * 128, hs], ot[:])

    layernorm(s2_sb, ln2_apply, sq2_pool, st2_pool)
    p7_ctx.close()
    s2_ctx.close()
    wde_ctx.close()
    gated_ctx.close()
    wg_ctx.close()
    s1_ctx.close()
    root.close()


# ---------------------------------------------------------------------------
_NC_CACHE = None


def _get_nc():
    global _NC_CACHE
    if _NC_CACHE is None:
        _NC_CACHE = build_program()
    return _NC_CACHE


def _to128(v, cols):
    """(cols*128,) vector -> [128, cols] with column c = v[c*128:(c+1)*128]."""
    return np.ascontiguousarray(np.asarray(v, np.float32).reshape(cols, 128).T)


def prep_inputs(inputs):
    hs = np.asarray(inputs["hidden_states"], np.float32)
    bias = np.asarray(inputs["bias"], np.float32)
    Wqkv = np.asarray(inputs["Wqkv"], np.float32)
    bqkv = np.asarray(inputs["bqkv"], np.float32)
    Wo = np.asarray(inputs["Wo"], np.float32)
    bo_v = np.asarray(inputs["bo"], np.float32)
    ln1_g = np.asarray(inputs["ln1_g"], np.float32)
    ln1_b = np.asarray(inputs["ln1_b"], np.float32)
    Wglu = np.asarray(inputs["Wglu"], np.float32)
    Wdown = np.asarray(inputs["Wdown"], np.float32)
    bdown = np.asarray(inputs["bdown"], np.float32)
    ln2_g = np.asarray(inputs["ln2_g"], np.float32)
    ln2_b = np.asarray(inputs["ln2_b"], np.float32)

    x_t = np.ascontiguousarray(hs.T)                       # (768, 8192)
    expbT = np.exp(np.ascontiguousarray(bias.transpose(0, 1, 3, 2))
                   ).astype(ml_dtypes.bfloat16)

    scale = 1.0 / np.sqrt(np.float32(HD))
    Wq = Wqkv.copy()
    Wq[:DIM] *= scale                                      # fold 1/sqrt(hd) into q
    wqkvT = np.ascontiguousarray(Wq.T)                     # (768, 2304)
    bqk_v = bqkv[:2 * DIM].copy()
    bqk_v[:DIM] *= scale
    bv_b = np.ascontiguousarray(
        np.broadcast_to(bqkv[2 * DIM:], (128, DIM)).astype(ml_dtypes.bfloat16))

    woT = np.ascontiguousarray(Wo.T)
    wgluT = np.ascontiguousarray((Wglu * ln1_g[None, :]).T)  # g1 folded
    cb1_v = Wglu @ ln1_b                                     # (6144,)
    c1_v = ln1_b + bdown
    wdownT = np.ascontiguousarray(Wdown.T).astype(ml_dtypes.bfloat16)

    shared = {
        "wqkvT": wqkvT,
        "bqk": _to128(bqk_v, 12),
        "bv_b": bv_b,
        "woT": woT,
        "bo": _to128(bo_v, 6),
        "wgluT": wgluT,
        "cb1": _to128(cb1_v, 48),
        "g1": _to128(ln1_g, 6),
        "c1": _to128(c1_v, 6),
        "wdownT": wdownT,
        "g2": _to128(ln2_g, 6),
        "b2": _to128(ln2_b, 6),
    }
    in_maps = []
    for c in range(N_CORES):
        m = dict(shared)
        m["xT"] = np.ascontiguousarray(x_t[:, c * N:(c + 1) * N])
        m["expbT"] = np.ascontiguousarray(expbT[c * NSEQ:(c + 1) * NSEQ])
        in_maps.append(m)
    return in_maps


def kernel(**inputs):
    nc = _get_nc()
    in_maps = prep_inputs(inputs)
    res = run_bass_kernel_spmd(nc, in_maps, core_ids=list(range(N_CORES)))
    outT = np.concatenate([res.results[c]["outT"] for c in range(N_CORES)],
                          axis=1)                          # (768, 8192)
    return np.ascontiguousarray(outT.T)



# revision 13
# speedup vs baseline: 1.0634x; 1.0634x over previous
"""BertAlibiLayer on 8 TRN2 NeuronCores — data-parallel over batch.

Per core: 2 sequences x 512 tokens, feature-major activations ([feature,
token]) so every matmul is transpose-free (weights pre-transposed on host).

v2 restructure vs. baseline:
- All projection matmuls run in bf16 (Fast Weight Load, half the SBUF read
  power => less HAM clock-throttle, half the weight DMA).
- Macro-pipeline over the two sequences: ACT-heavy attention(seq0) overlaps
  PE-heavy QKV(seq1); attention(seq1) overlaps Wdown(seq0). Gelu never
  interleaves with Exp/Ln so the ACT table set is stable within each window.
- Attention: softmax exp over paired [128,1024] PSUM tiles (halves the fixed
  ACT overhead), per-head software pipelining (scores/exp of head h emitted
  before ctx/normalize of head h-1), and the softmax denominator reciprocal
  computed by GPSIMD divide (off the ScalarE critical path).
- Q tiles pack two heads per [128,512] tile (rows 0:64 even head, 64:128 odd
  head); K tiles are half-zeroed so the K=128 scores contraction picks out
  one head. V blocks are 72 wide (64 features + ones + pad) instead of 128.
- SBUF pools run under the queue allocator; the two `side` stacks are used
  purely as two LIFO release stacks; big weight pools open at prologue so
  they take low ring addresses and the per-window pools churn above them.

Windows: A=qkv(s0) B=[attn(s0)|qkv(s1)] C=wo+ln1(s0) D=glu(s0)
         E=[attn(s1)|wdown(s0)] F=wo+ln1(s1),ln2(s0) G=glu(s1)
         H=wdown(s1)+ln2(s1)
"""

from contextlib import ExitStack

import numpy as np
import ml_dtypes

import concourse.bass as bass
import concourse.mybir as mybir
import concourse.tile as tile
from concourse import bacc
from concourse.bass_utils import run_bass_kernel_spmd

F32 = mybir.dt.float32
F32R = mybir.dt.float32r
BF16 = mybir.dt.bfloat16
AF = mybir.ActivationFunctionType
OP = mybir.AluOpType

DIM = 768
H = 12
HD = 64
S = 512
NSEQ = 2          # sequences per core
N = NSEQ * S      # tokens per core
I = 3072
KT = DIM // 128   # 6 k-tiles over DIM
VW = 72           # va block width per head: 64 features + ones col + pad
EPS = 1e-12
N_CORES = 8

OC_ORDER = [0, 6, 1, 7, 2, 8, 3, 9, 4, 10, 5, 11]  # q/k chunk emission order


def r(ap):
    """View an fp32 AP as float32r for full-rate PE streaming."""
    return ap.bitcast(F32R)


def build_program(gelu_func=AF.Gelu, nseqb=1):
    nc = bacc.Bacc("TRN2", target_bir_lowering=False, debug=False,
                   enable_asserts=False)
    # Steer the act-table chooser: the plain natural_log set lacks exp, so
    # Ln<->Exp sequences would reload tables every op. Emptying it (in place,
    # preserving set ids) makes the chooser use natural_log_exp_and_others.
    import concourse.hw_specs as hw_specs
    tabs = hw_specs.get_activation_tables(nc.m.arch)
    tabs["natural_log"] = set()

    # ---- DRAM parameters (per-core shards / replicated weights) ----
    xTb = nc.dram_tensor("xTb", [DIM, N], BF16, kind="ExternalInput").ap()
    expbT = nc.dram_tensor("expbT", [nseqb, H, S, S], BF16,
                           kind="ExternalInput").ap()
    wqkvT = nc.dram_tensor("wqkvT", [DIM, 3 * DIM], BF16,
                           kind="ExternalInput").ap()
    bqk = nc.dram_tensor("bqk", [128, 12], F32, kind="ExternalInput").ap()
    bv_b = nc.dram_tensor("bv_b", [128, DIM], BF16, kind="ExternalInput").ap()
    woT = nc.dram_tensor("woT", [DIM, DIM], BF16, kind="ExternalInput").ap()
    bo = nc.dram_tensor("bo", [128, 6], F32, kind="ExternalInput").ap()
    wgluT = nc.dram_tensor("wgluT", [DIM, 2 * I], BF16,
                           kind="ExternalInput").ap()
    cb1 = nc.dram_tensor("cb1", [128, 48], F32, kind="ExternalInput").ap()
    g1 = nc.dram_tensor("g1", [128, 6], F32, kind="ExternalInput").ap()
    c1 = nc.dram_tensor("c1", [128, 6], F32, kind="ExternalInput").ap()
    wdownT = nc.dram_tensor("wdownT", [I, DIM], BF16, kind="ExternalInput").ap()
    g2 = nc.dram_tensor("g2", [128, 6], F32, kind="ExternalInput").ap()
    b2 = nc.dram_tensor("b2", [128, 6], F32, kind="ExternalInput").ap()
    outT = nc.dram_tensor("outT", [DIM, N], F32, kind="ExternalOutput").ap()

    with tile.TileContext(nc, pool_alloc_mode="queue") as tc:
        emit(nc, tc, xTb, expbT, wqkvT, bqk, bv_b, woT, bo, wgluT, cb1,
             g1, c1, wdownT, g2, b2, outT, gelu_func, nseqb)

    nc.compile()
    return nc


def emit(nc, tc, xTb, expbT, wqkvT, bqk, bv_b, woT, bo, wgluT, cb1,
         g1, c1, wdownT, g2, b2, outT, gelu_func=AF.Gelu, nseqb=1):
    root = ExitStack()
    consts = root.enter_context(tc.tile_pool(name="consts", bufs=1, side="left"))
    # chain PSUM pool: projection matmul accumulators, open all kernel
    pch = root.enter_context(tc.tile_pool(name="pch", bufs=2, space="PSUM"))

    # Long-lived weight pools open first => low ring addresses, so the
    # per-window churn above them can reuse freed zones cleanly.
    wd_ctx = ExitStack()
    wd_pool = wd_ctx.enter_context(tc.tile_pool(name="wdown", bufs=24, side="left"))
    wg_ctx = ExitStack()
    wg_pool = wg_ctx.enter_context(tc.tile_pool(name="wglu", bufs=24, side="left"))
    wo_ctx = ExitStack()
    wo_pool = wo_ctx.enter_context(tc.tile_pool(name="wo", bufs=KT, side="left"))

    # ---------------- prologue: qkv weights + x ----------------
    wq_ctx = ExitStack()
    wq_pool = wq_ctx.enter_context(tc.tile_pool(name="wqkv", bufs=6, side="left"))
    xt_ctx = ExitStack()
    xtb_pool = xt_ctx.enter_context(tc.tile_pool(name="xtb", bufs=KT, side="left"))

    # critical path first: chain oc=0 needs wqk[kt][0] and xtb seq0 halves
    wqk_sb = [[None] * 12 for _ in range(KT)]
    for kt in range(KT):
        c = wq_pool.tile([128, 128], BF16, name=f"wqk{kt}_0", tag="wqk",
                         bufs=24)
        nc.sync.dma_start(c[:], wqkvT[kt * 128:(kt + 1) * 128, 0:128])
        wqk_sb[kt][0] = c
    xtb_sb = []
    for kt in range(KT):
        t = xtb_pool.tile([128, N], BF16, name=f"xtb{kt}", tag="xtb")
        nc.sync.dma_start(t[:, 0:512], xTb[kt * 128:(kt + 1) * 128, 0:512])
        xtb_sb.append(t)
    for kt in range(KT):
        c = wq_pool.tile([128, 128], BF16, name=f"wqk{kt}_6", tag="wqk",
                         bufs=24)
        nc.sync.dma_start(c[:], wqkvT[kt * 128:(kt + 1) * 128,
                                      6 * 128:7 * 128])
        wqk_sb[kt][6] = c

    # small constant tensors
    bqk_sb = consts.tile([128, 12], F32)
    nc.sync.dma_start(bqk_sb[:], bqk[:, :])
    bvb_sb = consts.tile([128, DIM], BF16)
    nc.sync.dma_start(bvb_sb[:], bv_b[:, :])
    bo_sb = consts.tile([128, 6], F32)
    nc.sync.dma_start(bo_sb[:], bo[:, :])
    cb1_sb = consts.tile([128, 48], F32)
    nc.sync.dma_start(cb1_sb[:], cb1[:, :])
    g1_sb = consts.tile([128, 6], F32)
    nc.sync.dma_start(g1_sb[:], g1[:, :])
    c1_sb = consts.tile([128, 6], F32)
    nc.sync.dma_start(c1_sb[:], c1[:, :])
    g2_sb = consts.tile([128, 6], F32)
    nc.sync.dma_start(g2_sb[:], g2[:, :])
    b2_sb = consts.tile([128, 6], F32)
    nc.sync.dma_start(b2_sb[:], b2[:, :])
    ones_f32c = consts.tile([128, 12], F32)
    nc.vector.memset(ones_f32c[:], 1.0)
    ones_col = consts.tile([128, 1], F32)   # stats lhsT: column of ones
    nc.vector.tensor_copy(ones_col[:].bitcast(F32R), ones_f32c[:, 0:1])
    ones_colb = consts.tile([128, 1], BF16)  # stats lhsT for bf16 src
    nc.vector.tensor_copy(ones_colb[:], ones_f32c[:, 0:1])
    ones_row = consts.tile([1, 128], F32)   # LN broadcast lhsT: row of ones
    nc.vector.memset(ones_row[:], 1.0)
    nc.vector.tensor_copy(ones_row[:].bitcast(F32R), ones_row[:])
    ones512 = consts.tile([1, 512], F32)    # numerator for gpsimd reciprocal
    nc.vector.memset(ones512[:], 1.0)
    eps_sb = consts.tile([1, 1], F32)
    nc.vector.memset(eps_sb[:], EPS)

    # rest of x, then remaining weight chunks in chain-consumption order
    for kt in range(KT):
        nc.sync.dma_start(xtb_sb[kt][:, 512:1024],
                          xTb[kt * 128:(kt + 1) * 128, 512:1024])
    for oc in OC_ORDER[2:]:
        for kt in range(KT):
            c = wq_pool.tile([128, 128], BF16, name=f"wqk{kt}_{oc}", tag="wqk",
                             bufs=24)
            nc.sync.dma_start(c[:], wqkvT[kt * 128:(kt + 1) * 128,
                                          oc * 128:(oc + 1) * 128])
            wqk_sb[kt][oc] = c
    wv_sb = []
    for kt in range(KT):
        v = wq_pool.tile([128, DIM], BF16, name=f"wv{kt}", tag="wv", bufs=6)
        nc.sync.dma_start(v[:], wqkvT[kt * 128:(kt + 1) * 128, 2 * DIM:])
        wv_sb.append(v)
    # big weights for later windows: queue after the qkv-critical stream
    wo_sb = []
    for kt in range(KT):
        t = wo_pool.tile([128, DIM], BF16, name=f"wo{kt}", tag="wo")
        nc.sync.dma_start(t[:], woT[kt * 128:(kt + 1) * 128, :])
        wo_sb.append(t)

    def load_wglu_group(g, phase):
        tiles = ([], [])
        for half in range(2):
            for kt in range(KT):
                t = wg_pool.tile([128, 1024], BF16,
                                 name=f"wg{half}_{g}_{kt}_{phase}", tag="wg")
                off = half * I + g * 1024
                nc.sync.dma_start(
                    t[:], wgluT[kt * 128:(kt + 1) * 128, off:off + 1024])
                tiles[half].append(t)
        return tiles

    wglu_groups = [None, None, None]
    wglu_groups[0] = load_wglu_group(0, 0)
    wglu_groups[1] = load_wglu_group(1, 0)
    wglu_groups[2] = load_wglu_group(2, 0)
    wd_sb = []
    for kt in range(24):
        t = wd_pool.tile([128, DIM], BF16, name=f"wd{kt}", tag="wd")
        nc.sync.dma_start(t[:], wdownT[kt * 128:(kt + 1) * 128, :])
        wd_sb.append(t)

    # ---------------- per-seq structures ----------------
    # qk_sb[s][0:6]  = packed q tiles (head 2i rows 0:64, head 2i+1 rows 64:128)
    # qk_sb[s][6:18] = half-zeroed k tiles (even head: rows 0:64 live;
    #                  odd head: rows 64:128 live)
    qk_ctx = [ExitStack(), ExitStack()]
    qk_sb = [None, None]
    va_ctx = [ExitStack(), ExitStack()]
    va_sb = [None, None]        # [s][sc] = [128, H*VW]

    def open_qkv_tiles(s, side):
        # seq1 setup lands mid-attention(seq0): run it on idle GPSIMD so the
        # DVE/ACT queues stay clear for the softmax pipeline
        eng = nc.vector if s == 0 else nc.gpsimd
        qp = qk_ctx[s].enter_context(
            tc.tile_pool(name=f"qk{s}", bufs=18, side=side))
        qk_sb[s] = [qp.tile([128, 512], BF16, name=f"qk{s}_{i}", tag="qk")
                    for i in range(18)]
        for h in range(H):
            kt_ = qk_sb[s][6 + h]
            if h % 2 == 0:
                eng.memset(kt_[64:128, :], 0.0)
            else:
                eng.memset(kt_[0:64, :], 0.0)
        vp = va_ctx[s].enter_context(
            tc.tile_pool(name=f"va{s}", bufs=4, side=side))
        va_sb[s] = []
        for sc in range(4):
            vt = vp.tile([128, H * VW], BF16, name=f"va{s}_{sc}", tag="va")
            va_sb[s].append(vt)
            vt_h = vt[:].rearrange("p (h c) -> p h c", c=VW)
            eng.memset(vt_h[:, :, HD + 1:], 0.0)
            eng.tensor_copy(vt_h[:, :, HD:HD + 1],
                            ones_f32c[:].rearrange("p (h c) -> p h c", c=1))

    def gen_qkv(s):
        """QKV projection + V assembly for sequence s. PE-heavy.

        seq1 re-DMAs each weight chunk through the ring: chunks are consumed
        once per allocation, so the 24-slot ring recycles without coupling
        window A to window B (HBM re-read is cheap)."""
        hs = slice(s * 512, (s + 1) * 512)
        for oc in OC_ORDER:
            if s == 1:
                for kt in range(KT):
                    c = wq_pool.tile([128, 128], BF16,
                                     name=f"wqk{kt}_{oc}_s1", tag="wqk",
                                     bufs=24)
                    nc.sync.dma_start(c[:], wqkvT[kt * 128:(kt + 1) * 128,
                                                  oc * 128:(oc + 1) * 128])
                    wqk_sb[kt][oc] = c
            ps = pch.tile([128, 512], F32, tag="ch")
            for kt in range(KT):
                nc.tensor.matmul(ps[:], wqk_sb[kt][oc][:], xtb_sb[kt][:, hs],
                                 start=(kt == 0), stop=(kt == KT - 1))
            if oc < 6:
                # q: both heads in one copy
                nc.vector.tensor_scalar(qk_sb[s][oc][:], ps[:],
                                        bqk_sb[:, oc:oc + 1], None,
                                        op0=OP.add)
            else:
                h0 = 2 * (oc - 6)
                nc.vector.tensor_scalar(qk_sb[s][6 + h0][0:64, :],
                                        ps[0:64, :], bqk_sb[0:64, oc:oc + 1],
                                        None, op0=OP.add)
                nc.scalar.activation(qk_sb[s][6 + h0 + 1][64:128, :],
                                     ps[64:128, :], AF.Identity,
                                     bias=bqk_sb[64:128, oc:oc + 1])
            yield
        for sc in range(4):
            vt = va_sb[s][sc]
            vt_h = vt[:].rearrange("p (h c) -> p h c", c=VW)
            gsc = s * 4 + sc
            for off, width, h0 in ((0, 512, 0), (512, 256, 8)):
                nh = width // HD
                ps = pch.tile([128, 512], F32, tag="ch")
                for kt in range(KT):
                    nc.tensor.matmul(
                        ps[:, :width],
                        xtb_sb[kt][:, gsc * 128:(gsc + 1) * 128],
                        wv_sb[kt][:, off:off + width],
                        start=(kt == 0), stop=(kt == KT - 1))
                nc.vector.tensor_add(
                    vt_h[:, h0:h0 + nh, 0:HD],
                    ps[:, :width].rearrange("p (h c) -> p h c", c=HD),
                    bvb_sb[:, off:off + width].rearrange("p (h c) -> p h c",
                                                         c=HD))
            yield

    # ---------------- attention ----------------
    def gen_attn(s, ctx_tiles, psc, pcp, tr_pool):
        """Attention for sequence s, software-pipelined by one head."""
        bseq = s if nseqb == 2 else 0

        def step1(h):
            q_tile = qk_sb[s][h // 2]
            k_tile = qk_sb[s][6 + h]
            e_tiles = []
            for pair in range(2):
                bt = tr_pool.tile([128, 2, 512], BF16, tag="bias", bufs=3)
                nc.gpsimd.dma_start(
                    bt[:], expbT[bseq, h, pair * 256:(pair + 1) * 256, :]
                    .rearrange("(c p) i -> p c i", p=128))
                pp = psc.tile([128, 1024], F32, tag="sc")
                for j in range(2):
                    jt = pair * 2 + j
                    nc.tensor.matmul(
                        pp[:, j * 512:(j + 1) * 512],
                        k_tile[:, jt * 128:(jt + 1) * 128],
                        q_tile[:], start=True, stop=True)
                st = tr_pool.tile([128, 1024], BF16, tag="sin", bufs=2)
                nc.scalar.activation(st[:], pp[:], AF.Exp)
                for j in range(2):
                    et = tr_pool.tile([128, 512], BF16, tag="exp", bufs=5)
                    nc.vector.tensor_mul(et[:], st[:, j * 512:(j + 1) * 512],
                                         bt[:, j, :])
                    e_tiles.append(et)
            return (h, e_tiles)

        def step2(p):
            h, e_tiles = p
            pc = pcp.tile([128, 512], F32, tag="pc")
            for jt in range(4):
                nc.tensor.matmul(pc[0:VW, :],
                                 va_sb[s][jt][:, h * VW:h * VW + VW],
                                 e_tiles[jt][:],
                                 start=(jt == 0), stop=(jt == 3))
            # 1/denom = exp(-ln(d)); Pool has no divide ucode, ACT pipelines
            # behind the next head's exp via the 1-head emission skew
            ld = tr_pool.tile([1, 512], F32, tag="dn", bufs=2)
            nc.scalar.activation(ld[:], pc[HD:HD + 1, :], AF.Ln)
            rc = tr_pool.tile([1, 512], F32, tag="rc", bufs=2)
            nc.scalar.activation(rc[:], ld[:], AF.Exp, scale=-1.0)
            bc = tr_pool.tile([64, 512], F32, tag="bc", bufs=2)
            nc.gpsimd.partition_broadcast(bc[:], rc[:], channels=64)
            nc.vector.tensor_mul(
                ctx_tiles[h // 2][(h % 2) * 64:(h % 2) * 64 + 64, :],
                pc[0:HD, :], bc[:])

        prev = step1(0)
        yield
        for h in range(1, H):
            cur = step1(h)
            step2(prev)      # finish head h-1
            prev = cur
            yield
        step2(prev)          # finish head 11
        yield

    # ---------------- building blocks ----------------
    def emit_wo(s, s1_tiles, ctx_tiles, xtf_tiles):
        for oc in range(KT):
            ps = pch.tile([128, 512], F32, tag="ch")
            for kt in range(KT):
                nc.tensor.matmul(ps[:], wo_sb[kt][:, oc * 128:(oc + 1) * 128],
                                 ctx_tiles[kt][:],
                                 start=(kt == 0), stop=(kt == KT - 1))
            nc.vector.scalar_tensor_tensor(
                s1_tiles[oc][:], ps[:], bo_sb[:, oc:oc + 1],
                xtf_tiles[oc][:], op0=OP.add, op1=OP.add)

    def layernorm(src, pst, pbc, sq_pool, stat_pool, apply_cb):
        """Feature-axis layernorm over KT [128,512] bf16 tiles."""
        psx_t = pst.tile([1, 512], F32, tag="st", name="psx")
        psxx_t = pst.tile([1, 512], F32, tag="st", name="psxx")
        psx = psx_t[:]
        psxx = psxx_t[:]
        for oc in range(KT):
            sq = sq_pool.tile([128, 512], F32, tag="sq")
            nc.scalar.activation(sq[:].bitcast(F32R), src[oc][:], AF.Square)
            nc.tensor.matmul(psx, ones_colb[:], src[oc][:],
                             start=(oc == 0), stop=(oc == KT - 1))
            nc.tensor.matmul(psxx, r(ones_col[:]), r(sq[:]),
                             start=(oc == 0), stop=(oc == KT - 1))
        m_sb = stat_pool.tile([1, 512], F32, tag="st")
        nc.scalar.activation(m_sb[:], psx, AF.Identity, scale=1.0 / DIM)
        msq = stat_pool.tile([1, 512], F32, tag="st")
        nc.scalar.activation(msq[:], psx, AF.Square, scale=1.0 / DIM)
        var = stat_pool.tile([1, 512], F32, tag="st")
        nc.vector.scalar_tensor_tensor(var[:], psxx, 1.0 / DIM, msq[:],
                                       op0=OP.mult, op1=OP.subtract)
        lv = stat_pool.tile([1, 512], F32, tag="st")
        nc.scalar.activation(lv[:], var[:], AF.Ln, bias=eps_sb[:1, :1])
        rs = stat_pool.tile([1, 512], F32, tag="st")
        nc.scalar.activation(rs[:], lv[:], AF.Exp, scale=-0.5)
        m_r = stat_pool.tile([1, 512], F32, tag="st")
        nc.vector.tensor_copy(m_r[:].bitcast(F32R), m_sb[:])
        rs_r = stat_pool.tile([1, 512], F32, tag="st")
        nc.vector.tensor_copy(rs_r[:].bitcast(F32R), rs[:])
        mbc = pbc.tile([128, 512], F32, tag="bc")
        nc.tensor.matmul(mbc[:], r(ones_row[:]), r(m_r[:]),
                         start=True, stop=True)
        rbc = pbc.tile([128, 512], F32, tag="bc")
        nc.tensor.matmul(rbc[:], r(ones_row[:]), r(rs_r[:]),
                         start=True, stop=True)
        for oc in range(KT):
            apply_cb(oc, mbc, rbc)

    def emit_ln1(s, s1_tiles, z1_tiles, r_tiles, pst, pbc, sq_pool, stat_pool,
                 t_pool):
        def apply(oc, mbc, rbc):
            t = t_pool.tile([128, 512], F32, tag="t")
            nc.vector.tensor_sub(t[:], s1_tiles[oc][:], mbc[:])
            nc.vector.tensor_mul(z1_tiles[oc][:], t[:], rbc[:])
            # residual carry r = z1*g1 + (ln1_b + bdown) on idle GPSIMD
            nc.gpsimd.tensor_scalar(r_tiles[oc][:], z1_tiles[oc][:],
                                    g1_sb[:, oc:oc + 1], c1_sb[:, oc:oc + 1],
                                    op0=OP.mult, op1=OP.add)

        layernorm(s1_tiles, pst, pbc, sq_pool, stat_pool, apply)

    def gen_glu(s, z1_tiles, gated_tiles, groups, group_order, pglu,
                gelu_pool):
        for g in group_order:
            wg1, wg2 = groups[g]
            for j in range(8):
                oc = g * 8 + j
                ps = pglu.tile([128, 512], F32, tag="gl")
                for kt in range(KT):
                    nc.tensor.matmul(
                        ps[:], wg1[kt][:, j * 128:(j + 1) * 128],
                        z1_tiles[kt][:],
                        start=(kt == 0), stop=(kt == KT - 1))
                ge = gelu_pool.tile([128, 512], F32, tag="gelu")
                nc.scalar.activation(ge[:], ps[:], gelu_func,
                                     bias=cb1_sb[:, oc:oc + 1])
                ps2 = pch.tile([128, 512], F32, tag="ch")
                for kt in range(KT):
                    nc.tensor.matmul(
                        ps2[:], wg2[kt][:, j * 128:(j + 1) * 128],
                        z1_tiles[kt][:],
                        start=(kt == 0), stop=(kt == KT - 1))
                nc.vector.scalar_tensor_tensor(
                    gated_tiles[oc][:], ps2[:], cb1_sb[:, 24 + oc:25 + oc],
                    ge[:], op0=OP.add, op1=OP.mult)
                yield

    def gen_wdown(s, gated_tiles, r_tiles, s2_tiles):
        for oc in range(KT):
            ps = pch.tile([128, 512], F32, tag="ch")
            for kt in range(12):
                nc.tensor.matmul(ps[:], wd_sb[kt][:, oc * 128:(oc + 1) * 128],
                                 gated_tiles[kt][:],
                                 start=(kt == 0), stop=False)
            yield
            for kt in range(12, 24):
                nc.tensor.matmul(ps[:], wd_sb[kt][:, oc * 128:(oc + 1) * 128],
                                 gated_tiles[kt][:],
                                 start=False, stop=(kt == 23))
            nc.vector.tensor_add(s2_tiles[oc][:], ps[:], r_tiles[oc][:])
            yield

    def emit_ln2(s, s2_tiles, pst, pbc, sq_pool, stat_pool, t_pool, out_pool):
        def apply(oc, mbc, rbc):
            t = t_pool.tile([128, 512], F32, tag="t")
            nc.vector.tensor_sub(t[:], s2_tiles[oc][:], mbc[:])
            zz = t_pool.tile([128, 512], F32, tag="zz")
            nc.vector.scalar_tensor_tensor(zz[:], t[:], g2_sb[:, oc:oc + 1],
                                           rbc[:], op0=OP.mult, op1=OP.mult)
            ot = out_pool.tile([128, 512], F32, tag="out")
            nc.vector.tensor_scalar(ot[:], zz[:], b2_sb[:, oc:oc + 1], None,
                                    op0=OP.add)
            nc.gpsimd.dma_start(outT[oc * 128:(oc + 1) * 128,
                                     s * 512:(s + 1) * 512], ot[:])

        layernorm(s2_tiles, pst, pbc, sq_pool, stat_pool, apply)

    def drain(gen):
        for _ in gen:
            pass

    def interleave(ga, gb):
        """Alternate emission between two generators until both exhaust."""
        a_live = b_live = True
        while a_live or b_live:
            if a_live:
                try:
                    next(ga)
                except StopIteration:
                    a_live = False
            if b_live:
                try:
                    next(gb)
                except StopIteration:
                    b_live = False

    def tiles6(pool, nm, dt):
        return [pool.tile([128, 512], dt, name=f"{nm}_{oc}", tag=nm)
                for oc in range(KT)]

    def load_xtf(pool, s):
        xtf = []
        for kt in range(KT):
            t = pool.tile([128, 512], BF16, name=f"xtf{s}_{kt}", tag="xtf")
            nc.sync.dma_start(t[:], xTb[kt * 128:(kt + 1) * 128,
                                        s * 512:(s + 1) * 512])
            xtf.append(t)
        return xtf

    # ======================= schedule =======================
    # --- A: QKV(s0) ---  [L above xtb: qk0, va0]
    open_qkv_tiles(0, "left")
    drain(gen_qkv(0))

    # --- B: attn(s0) || QKV(s1) ---
    # R stack bottom-up: qk1, va1, tr (lives to E-end), ctx0
    open_qkv_tiles(1, "right")
    tr_ctx = ExitStack()
    tr_pool = tr_ctx.enter_context(tc.tile_pool(name="tr", bufs=2, side="right"))
    ctx0_ctx = ExitStack()
    ctx0 = tiles6(ctx0_ctx.enter_context(tc.tile_pool(name="ctx0", bufs=KT,
                                                      side="right")), "c0", BF16)
    attn_ps = ExitStack()
    psc = attn_ps.enter_context(tc.tile_pool(name="psc0", bufs=2, space="PSUM"))
    pcp = attn_ps.enter_context(tc.tile_pool(name="pc0", bufs=2, space="PSUM"))
    interleave(gen_attn(0, ctx0, psc, pcp, tr_pool), gen_qkv(1))
    attn_ps.close()
    va_ctx[0].close()
    qk_ctx[0].close()
    xt_ctx.close()
    wq_ctx.close()

    # --- C: Wo(s0) + LN1(s0) ---
    # L open order (reverse close): r0 (lives to F), s1_0, z1_0 (to D), then
    # C-local xtf0/sq/stat/t
    r0_ctx = ExitStack()
    r0 = tiles6(r0_ctx.enter_context(tc.tile_pool(name="r0", bufs=KT,
                                                  side="left")), "r0", BF16)
    s1_0_ctx = ExitStack()
    s1_0 = tiles6(s1_0_ctx.enter_context(tc.tile_pool(name="s1_0", bufs=KT,
                                                      side="left")), "s10", BF16)
    z1_0_ctx = ExitStack()
    z1_0 = tiles6(z1_0_ctx.enter_context(tc.tile_pool(name="z1_0", bufs=KT,
                                                      side="left")), "z10", BF16)
    cwin = ExitStack()
    xtf0 = load_xtf(cwin.enter_context(tc.tile_pool(name="xtf0", bufs=KT,
                                                    side="left")), 0)
    sq_pool = cwin.enter_context(tc.tile_pool(name="sq0", bufs=3, side="left"))
    stat_pool = cwin.enter_context(tc.tile_pool(name="stat0", bufs=3, side="left"))
    t_pool = cwin.enter_context(tc.tile_pool(name="t0", bufs=3, side="left"))
    ln_ps = ExitStack()
    pst = ln_ps.enter_context(tc.tile_pool(name="pst0", bufs=2, space="PSUM"))
    pbc = ln_ps.enter_context(tc.tile_pool(name="pbc0", bufs=2, space="PSUM"))
    emit_wo(0, s1_0, ctx0, xtf0)
    emit_ln1(0, s1_0, z1_0, r0, pst, pbc, sq_pool, stat_pool, t_pool)
    ln_ps.close()
    cwin.close()
    ctx0_ctx.close()   # R top

    # --- D: GLU(s0) ---  [R: gated0; L: gelu0]
    gated0_ctx = ExitStack()
    gated0_pool = gated0_ctx.enter_context(
        tc.tile_pool(name="gated0", bufs=24, side="right"))
    gated0 = [gated0_pool.tile([128, 512], BF16, name=f"g0_{j}", tag="gated")
              for j in range(24)]
    dwin = ExitStack()
    gelu_pool = dwin.enter_context(tc.tile_pool(name="gelu0", bufs=3, side="left"))
    glu_ps = ExitStack()
    pglu = glu_ps.enter_context(tc.tile_pool(name="pglu0", bufs=2, space="PSUM"))
    drain(gen_glu(0, z1_0, gated0, wglu_groups, [0, 1, 2], pglu, gelu_pool))
    glu_ps.close()
    dwin.close()
    z1_0_ctx.close()
    s1_0_ctx.close()

    # --- E: attn(s1) || Wdown(s0) ---  [L: ctx1, s2_0]
    ctx1_ctx = ExitStack()
    ctx1 = tiles6(ctx1_ctx.enter_context(tc.tile_pool(name="ctx1", bufs=KT,
                                                      side="left")), "c1", BF16)
    s2_0_ctx = ExitStack()
    s2_0 = tiles6(s2_0_ctx.enter_context(tc.tile_pool(name="s2_0", bufs=KT,
                                                      side="left")), "s20", BF16)
    attn_ps = ExitStack()
    psc = attn_ps.enter_context(tc.tile_pool(name="psc1", bufs=2, space="PSUM"))
    pcp = attn_ps.enter_context(tc.tile_pool(name="pc1", bufs=2, space="PSUM"))
    interleave(gen_attn(1, ctx1, psc, pcp, tr_pool),
               gen_wdown(0, gated0, r0, s2_0))
    attn_ps.close()
    gated0_ctx.close()
    tr_ctx.close()
    va_ctx[1].close()
    qk_ctx[1].close()
    # reload wglu group 0 for G (group 2 stays resident)
    wglu_groups[0] = load_wglu_group(0, 1)

    # --- F: Wo(s1) + LN1(s1), then LN2(s0) + out(s0) ---
    # R open order: r1, gated1 (both live to H), s1_1, z1_1 (to G), F-locals
    r1_ctx = ExitStack()
    r1 = tiles6(r1_ctx.enter_context(tc.tile_pool(name="r1", bufs=KT,
                                                  side="right")), "r1", BF16)
    gated1_ctx = ExitStack()
    gated1_pool = gated1_ctx.enter_context(
        tc.tile_pool(name="gated1", bufs=24, side="right"))
    gated1 = [gated1_pool.tile([128, 512], BF16, name=f"g1_{j}", tag="gated")
              for j in range(24)]
    s1_1_ctx = ExitStack()
    s1_1 = tiles6(s1_1_ctx.enter_context(tc.tile_pool(name="s1_1", bufs=KT,
                                                      side="right")), "s11", BF16)
    z1_1_ctx = ExitStack()
    z1_1 = tiles6(z1_1_ctx.enter_context(tc.tile_pool(name="z1_1", bufs=KT,
                                                      side="right")), "z11", BF16)
    fwin = ExitStack()
    xtf1 = load_xtf(fwin.enter_context(tc.tile_pool(name="xtf1", bufs=KT,
                                                    side="right")), 1)
    sq_pool = fwin.enter_context(tc.tile_pool(name="sq1", bufs=3, side="right"))
    stat_pool = fwin.enter_context(tc.tile_pool(name="stat1", bufs=6, side="right"))
    t_pool = fwin.enter_context(tc.tile_pool(name="t1", bufs=3, side="right"))
    out_pool = fwin.enter_context(tc.tile_pool(name="outp0", bufs=3, side="right"))
    ln_ps = ExitStack()
    pst = ln_ps.enter_context(tc.tile_pool(name="pst1", bufs=4, space="PSUM"))
    pbc = ln_ps.enter_context(tc.tile_pool(name="pbc1", bufs=2, space="PSUM"))
    emit_wo(1, s1_1, ctx1, xtf1)
    emit_ln1(1, s1_1, z1_1, r1, pst, pbc, sq_pool, stat_pool, t_pool)
    emit_ln2(0, s2_0, pst, pbc, sq_pool, stat_pool, t_pool, out_pool)
    ln_ps.close()
    fwin.close()
    s2_0_ctx.close()
    ctx1_ctx.close()
    r0_ctx.close()
    wo_ctx.close()
    # reload wglu group 1 for G
    wglu_groups[1] = load_wglu_group(1, 1)

    # --- G: GLU(s1) --- group 2 first (still resident from D)
    dwin = ExitStack()
    gelu_pool = dwin.enter_context(tc.tile_pool(name="gelu1", bufs=3, side="left"))
    glu_ps = ExitStack()
    pglu = glu_ps.enter_context(tc.tile_pool(name="pglu1", bufs=2, space="PSUM"))
    drain(gen_glu(1, z1_1, gated1, wglu_groups, [2, 0, 1], pglu, gelu_pool))
    glu_ps.close()
    dwin.close()
    z1_1_ctx.close()
    s1_1_ctx.close()
    wg_ctx.close()

    # --- H: Wdown(s1) + LN2(s1) + out(s1) ---
    s2_1_ctx = ExitStack()
    s2_1 = tiles6(s2_1_ctx.enter_context(tc.tile_pool(name="s2_1", bufs=KT,
                                                      side="left")), "s21", BF16)
    hwin = ExitStack()
    sq_pool = hwin.enter_context(tc.tile_pool(name="sq2", bufs=3, side="left"))
    stat_pool = hwin.enter_context(tc.tile_pool(name="stat2", bufs=3, side="left"))
    t_pool = hwin.enter_context(tc.tile_pool(name="t2", bufs=3, side="left"))
    out_pool = hwin.enter_context(tc.tile_pool(name="outp1", bufs=3, side="left"))
    ln_ps = ExitStack()
    pst = ln_ps.enter_context(tc.tile_pool(name="pst2", bufs=2, space="PSUM"))
    pbc = ln_ps.enter_context(tc.tile_pool(name="pbc2", bufs=2, space="PSUM"))
    drain(gen_wdown(1, gated1, r1, s2_1))
    emit_ln2(1, s2_1, pst, pbc, sq_pool, stat_pool, t_pool, out_pool)
    ln_ps.close()
    hwin.close()
    s2_1_ctx.close()
    gated1_ctx.close()
    r1_ctx.close()
    wd_ctx.close()
    root.close()


# ---------------------------------------------------------------------------
_NC_CACHE = {}


def _get_nc(nseqb=1):
    if nseqb not in _NC_CACHE:
        _NC_CACHE[nseqb] = build_program(nseqb=nseqb)
    return _NC_CACHE[nseqb]


def _to128(v, cols):
    """(cols*128,) vector -> [128, cols] with column c = v[c*128:(c+1)*128]."""
    return np.ascontiguousarray(np.asarray(v, np.float32).reshape(cols, 128).T)


def prep_inputs(inputs):
    hs = np.asarray(inputs["hidden_states"], np.float32)
    bias = np.asarray(inputs["bias"], np.float32)
    Wqkv = np.asarray(inputs["Wqkv"], np.float32)
    bqkv = np.asarray(inputs["bqkv"], np.float32)
    Wo = np.asarray(inputs["Wo"], np.float32)
    bo_v = np.asarray(inputs["bo"], np.float32)
    ln1_g = np.asarray(inputs["ln1_g"], np.float32)
    ln1_b = np.asarray(inputs["ln1_b"], np.float32)
    Wglu = np.asarray(inputs["Wglu"], np.float32)
    Wdown = np.asarray(inputs["Wdown"], np.float32)
    bdown = np.asarray(inputs["bdown"], np.float32)
    ln2_g = np.asarray(inputs["ln2_g"], np.float32)
    ln2_b = np.asarray(inputs["ln2_b"], np.float32)

    x_t = np.ascontiguousarray(hs.T)                       # (768, 8192)
    x_tb = x_t.astype(ml_dtypes.bfloat16)

    # batch-broadcast bias (alibi) => ship one copy per core, reuse per seq
    batch_const = bool((bias[1:] == bias[:1]).all())
    nseqb = 1 if batch_const else 2
    if batch_const:
        expbT = np.exp(np.ascontiguousarray(
            bias[0].transpose(0, 2, 1)))[None].astype(ml_dtypes.bfloat16)
    else:
        expbT = np.exp(np.ascontiguousarray(bias.transpose(0, 1, 3, 2))
                       ).astype(ml_dtypes.bfloat16)

    scale = 1.0 / np.sqrt(np.float32(HD))
    Wq = Wqkv.copy()
    Wq[:DIM] *= scale                                      # fold 1/sqrt(hd) into q
    wqkvT = np.ascontiguousarray(Wq.T).astype(ml_dtypes.bfloat16)
    bqk_v = bqkv[:2 * DIM].copy()
    bqk_v[:DIM] *= scale
    bv_b = np.ascontiguousarray(
        np.broadcast_to(bqkv[2 * DIM:], (128, DIM)).astype(ml_dtypes.bfloat16))

    woT = np.ascontiguousarray(Wo.T).astype(ml_dtypes.bfloat16)
    wgluT = np.ascontiguousarray((Wglu * ln1_g[None, :]).T
                                 ).astype(ml_dtypes.bfloat16)  # g1 folded
    cb1_v = Wglu @ ln1_b                                     # (6144,)
    c1_v = ln1_b + bdown
    wdownT = np.ascontiguousarray(Wdown.T).astype(ml_dtypes.bfloat16)

    shared = {
        "wqkvT": wqkvT,
        "bqk": _to128(bqk_v, 12),
        "bv_b": bv_b,
        "woT": woT,
        "bo": _to128(bo_v, 6),
        "wgluT": wgluT,
        "cb1": _to128(cb1_v, 48),
        "g1": _to128(ln1_g, 6),
        "c1": _to128(c1_v, 6),
        "wdownT": wdownT,
        "g2": _to128(ln2_g, 6),
        "b2": _to128(ln2_b, 6),
    }
    in_maps = []
    for c in range(N_CORES):
        m = dict(shared)
        m["xTb"] = np.ascontiguousarray(x_tb[:, c * N:(c + 1) * N])
        if batch_const:
            m["expbT"] = expbT
        else:
            m["expbT"] = np.ascontiguousarray(expbT[c * NSEQ:(c + 1) * NSEQ])
        in_maps.append(m)
    return in_maps, nseqb


def kernel(**inputs):
    in_maps, nseqb = prep_inputs(inputs)
    nc = _get_nc(nseqb)
    res = run_bass_kernel_spmd(nc, in_maps, core_ids=list(range(N_CORES)))
    outT = np.concatenate([res.results[c]["outT"] for c in range(N_CORES)],
                          axis=1)                          # (768, 8192)
    return np.ascontiguousarray(outT.T)
